# revision 34
# baseline (speedup 1.0000x reference)
"""CTC loss (keras ctc_batch_cost port, input_len=C source bug replicated)
on 8 Trainium2 NeuronCores.

Strategy
--------
Data parallel over batch: 512 samples -> 64 per core.

The alpha recursion is run in *linear probability space* with periodic
per-sample rescaling (classic scaled CTC forward pass) instead of log-space
logsumexp: each step is then only shift-adds and multiplies.

The 127-step serial chain is cut in half: the forward recursion covers
t=1..63 while the *backward* (transposed) recursion covers t=127..64,
computed simultaneously.  After reversing the state axis of the backward
chain, both recursions have the identical shift structure

    X' = (X + sh1(X) + mask . sh2(X)) * Q_t

so one [128 x 129] tile holds both: partitions 0..63 = forward states of the
64 samples, partitions 64..127 = (reversed) backward states.  63 joint steps
replace 127, and all 128 vector lanes are busy.

The host pre-gathers Q[b,t,s] = y_pred[b,t,ext[b,s]] + eps, prescaled by the
per-(b,t) max (its log is re-added on the host at the end), so device values
stay O(1) and only 12 cheap renorms are needed; each renorm's scale factor is
the free accum_out (row sum) of that step's scalar_tensor_tensor, and its
reciprocal folds into the next step's multiply, so renorm adds no extra
full-width ops.  The device ships the final joint state X and the renorm
factors; the host does the tiny junction contraction and all logs in float64:

    tail[b] = sum_s (T A_63)[b,s] * U_64[b,s]
    loss[b] = -( log tail[b] + sum_t log M[b,t] + sum_j log r[b,j] )

Device instruction budget per joint step: 3x tensor_tensor (shift-add full
width, mask-mul + add on odd states only) + 1x scalar_tensor_tensor
(x rescale x Q), all on VectorE - measured ~63us VectorE-busy per core,
~91us total NEFF time including DMA ramp and Tile's closing barrier.
"""

import os
import numpy as np

import concourse.bass as bass
import concourse.tile as tile
from concourse import mybir
from concourse.bass_utils import run_bass_kernel_spmd

# Problem constants (nn_CTCLayer: B,T,C,L = 512,512,128,64)
B, T, C, L = 512, 512, 128, 64
TU = C                    # input_len = y_pred.shape[2] (source bug, replicated)
S = 2 * L + 1             # 129 extended states
SP = 132                  # padded state width (pad cols are zero)
NSTEP = (TU - 2) // 2     # 63 joint fwd/bwd steps
NCORE = 8
BL = B // NCORE           # 64 samples per core
KRE = 5                   # renorm period (worst-case 5-step shrink 1e-35 > f32 min)
NRE = 12                  # renorms at joint steps 5,10,...,60
CHUNKS = [3, 6, 6, 8, 8, 8, 8, 8, 8]   # q-DMA chunk sizes (steps); small first
EPS = np.float32(1e-7)

LAST_RESULTS = None       # test harness peeks at this for profiling info


def _build_bass(niter=1, hwdge=True, gps_mul=False):
    """niter>1 repeats the full computation (re-init each time) so host-side
    timing of T(niter)-T(1) isolates pure device execution time."""
    nc = bass.Bass()
    q_d = nc.declare_dram_parameter(
        "q", [128, NSTEP * SP], mybir.dt.float32, isOutput=False)
    init_d = nc.declare_dram_parameter(
        "init", [128, 2 + SP], mybir.dt.float32, isOutput=False)
    mask_d = nc.declare_dram_parameter(
        "maskodd", [128, 64], mybir.dt.float32, isOutput=False)
    xout_d = nc.declare_dram_parameter(
        "xout", [128, 2 + SP], mybir.dt.float32, isOutput=True)
    rmax_d = nc.declare_dram_parameter(
        "rmaxs", [128, 16], mybir.dt.float32, isOutput=True)

    mult = mybir.AluOpType.mult

    with tile.TileContext(nc) as tc, tc.tile_pool(name="p", bufs=1) as pool:
        # Small tensors ride HWDGE (cheap ~650ns trigger, tiny transfer);
        # the 4.2MB q stream rides SWDGE (313GB/s measured, vs ~55GB/s for
        # HWDGE here), chunked so step 1 only waits for the first small chunk.
        mk = pool.tile([128, 64], mybir.dt.float32, tag="mk")
        nc.scalar.dma_start(mk[:, :], mask_d[:, :])

        qt = []
        step_of = []                    # step index (0-based) -> (chunk, k)
        for ci, csz in enumerate(CHUNKS):
            t = pool.tile([128, csz * SP], mybir.dt.float32, tag=f"q{ci}")
            qt.append(t)
            for k in range(csz):
                step_of.append((ci, k))
        assert len(step_of) == NSTEP

        xc = pool.tile([128, 2 + SP], mybir.dt.float32, tag="xc")
        t1 = pool.tile([128, 130], mybir.dt.float32, tag="t1")
        g = pool.tile([128, 64], mybir.dt.float32, tag="g")
        rmx = pool.tile([128, 16], mybir.dt.float32, tag="rmx")
        rin = pool.tile([128, 16], mybir.dt.float32, tag="rin")
        nc.vector.memset(rmx[:, :], 0.0)

        for it in range(niter):
            nc.scalar.dma_start(xc[:, :], init_d[:, :])
            if it == 0:
                off = 0
                for ci, csz in enumerate(CHUNKS):
                    nc.gpsimd.dma_start(
                        qt[ci][:, :], q_d[:, off * SP:(off + csz) * SP])
                    off += csz
            for i in range(1, NSTEP + 1):
                ci, k = step_of[i - 1]
                qi = qt[ci][:, k * SP:k * SP + S]
                eng_mul = nc.gpsimd if gps_mul else nc.vector
                eng_mul.tensor_mul(g[:, :], mk[:, :], xc[:, 1:S:2])
                nc.vector.tensor_add(t1[:, 0:S], xc[:, 2:2 + S], xc[:, 1:1 + S])
                nc.vector.tensor_add(t1[:, 1:S:2], t1[:, 1:S:2], g[:, :])
                if i >= 2 and (i - 1) % KRE == 0:
                    sc = rin[:, (i - 1) // KRE - 1:(i - 1) // KRE]
                else:
                    sc = 1.0
                # every KRE-th step: fused accum_out gives sum_s X'[s], used
                # as the rescale factor (any positive per-sample scale works)
                ac = (rmx[:, i // KRE - 1:i // KRE]
                      if (i % KRE == 0 and i <= NRE * KRE) else None)
                nc.vector.scalar_tensor_tensor(
                    xc[:, 2:2 + S], t1[:, 0:S], sc, qi, mult, mult,
                    accum_out=ac)
                if ac is not None:
                    nc.vector.reciprocal(
                        rin[:, i // KRE - 1:i // KRE], ac)

        # junction (tail = U_64^T T A_63) moved to the host in f64: ship the
        # final joint state X and the renorm factors, nothing else to compute
        nc.scalar.dma_start(xout_d[:, :], xc[:, :])
        nc.scalar.dma_start(rmax_d[:, :], rmx[:, :])
    _split_excess_waits(nc)
    return nc


def _split_excess_waits(nc):
    """This walrus build allows only ONE sync wait per instruction encoding
    (see bass_rust.inst_waits_full).  Tile still emits a few instructions with
    more (the closing Drain, DMAs with producer+ring waits).  Hoist the excess
    waits onto same-engine NoOps inserted just before the instruction —
    program order on the engine queue makes this semantically identical."""
    ctr = [0]
    for f in nc.m.functions:
        for blk in f.blocks:
            il = blk.instructions
            out = []
            changed = False
            for inst in il:
                si = inst.sync_info
                if si is not None and si.on_wait and len(si.on_wait) > 1:
                    waits = list(si.on_wait)
                    for w in waits[:-1]:
                        nop = mybir.InstNoOp(
                            name=f"waitnop_{ctr[0]}", ins=[], outs=[])
                        ctr[0] += 1
                        nop.engine = inst.engine
                        nop.sync_info = mybir.SyncInfo(
                            on_wait=[w], on_update=[])
                        out.append(nop)
                    inst.sync_info = mybir.SyncInfo(
                        on_wait=[waits[-1]], on_update=list(si.on_update or []))
                    changed = True
                out.append(inst)
            if changed:
                blk.instructions = out


def _host_prep(y_true, y_pred):
    """Gather/prescale P-hat, masks, per-core device inputs, host log sums."""
    yp = np.asarray(y_pred, dtype=np.float32)[:, :TU, :]
    yt = np.asarray(y_true)
    blank = C - 1

    ext = np.full((B, S), blank, dtype=np.int64)
    ext[:, 1::2] = yt
    P = np.take_along_axis(yp, ext[:, None, :], axis=2) + EPS     # [B,TU,S]
    M = P.max(axis=2)                                             # [B,TU]
    Phat = (P / M[:, :, None]).astype(np.float32)
    logM = np.log(M.astype(np.float64)).sum(axis=1)               # [B] f64

    mask_f = np.zeros((B, S), dtype=np.float32)
    mask_f[:, 3::2] = (yt[:, 1:] != yt[:, :-1]).astype(np.float32)
    mask_r = np.zeros((B, S), dtype=np.float32)
    mask_r[:, 2:S] = mask_f[:, S - 1:1:-1]    # mask_r[sh] = mask_f[S+1-sh]

    in_maps = []
    for c in range(NCORE):
        bs = slice(c * BL, (c + 1) * BL)
        qh = np.zeros((128, NSTEP, SP), dtype=np.float32)
        qh[0:BL, :, 0:S] = Phat[bs, 1:NSTEP + 1, :]
        qh[BL:128, :, 0:S] = Phat[bs, TU - 2:TU - 2 - NSTEP:-1, ::-1]
        init = np.zeros((128, 2 + SP), dtype=np.float32)
        init[0:BL, 2] = Phat[bs, 0, 0]
        init[0:BL, 3] = Phat[bs, 0, 1]
        init[BL:128, 2] = Phat[bs, TU - 1, S - 1]
        init[BL:128, 3] = Phat[bs, TU - 1, S - 2]
        maskodd = np.zeros((128, 64), dtype=np.float32)
        maskodd[0:BL, :] = mask_f[bs, 1::2]
        maskodd[BL:128, :] = mask_r[bs, 1::2]
        in_maps.append({
            "q": np.ascontiguousarray(qh.reshape(128, NSTEP * SP)),
            "init": init,
            "maskodd": maskodd,
        })
    return in_maps, logM, mask_f


def _finish_host(out, logM_c, mask_f_c):
    """Junction + logs in float64: tail = U_64^T (T A_63), per core."""
    X = out["xout"].astype(np.float64)
    A, V = X[0:BL, 2:2 + S], X[BL:128, 2:2 + S]
    TA = A.copy()
    TA[:, 1:] += A[:, :-1]
    TA[:, 2:] += mask_f_c[:, 2:] * A[:, :-2]
    tail = (TA * V[:, ::-1]).sum(axis=1)
    lacc = np.log(out["rmaxs"][:, :NRE].astype(np.float64)).sum(axis=1)
    return -(np.log(tail) + logM_c + lacc[0:BL] + lacc[BL:128])


def kernel(y_true, y_pred):
    global LAST_RESULTS
    in_maps, logM, mask_f = _host_prep(y_true, y_pred)
    nc = _build_bass()
    trace = os.environ.get("CTC_TRACE", "0") == "1"
    res = None
    for attempt in range(3):
        try:
            res = run_bass_kernel_spmd(
                nc, in_maps, list(range(NCORE)), trace=trace)
            break
        except Exception:
            # the axon-tunneled device occasionally reports a transient
            # NRT_EXEC_UNIT_UNRECOVERABLE; a retry on a fresh build recovers
            if attempt == 2:
                raise
            import time
            time.sleep(20)
            nc = _build_bass()
    LAST_RESULTS = res

    loss = np.empty((B,), dtype=np.float64)
    for c in range(NCORE):
        bs = slice(c * BL, (c + 1) * BL)
        loss[bs] = _finish_host(
            res.results[c], logM[bs], mask_f[bs].astype(np.float64))
    return loss.reshape(B, 1).astype(np.float32)
